# revision 18
# baseline (speedup 1.0000x reference)
"""HPC-RNN (hippocampus model) forward pass on 8 Trainium2 NeuronCores.

Strategy: pure data parallelism. batch=128 is split 16 per core; all weights
are replicated. Everything on-device lives in a transposed "neuron-partition"
layout so that the per-step recurrence's elementwise work runs on all 128
DVE/ACT lanes:

    state tile [128, 64]:  tile[p, jb*16 + b] = X^T[jb*128 + p, b]

(4 j-blocks of 128 neurons stacked along the free dim, 16 batch columns each.)

Per step (T=512 steps, strictly sequential recurrence):
    z1   = ec3 @ W1                 16 matmuls (K=128 x 4, M=128 x 4, N=16)
    ca1  = clip(drive*(1+sig(z1)) - ca1bias, 0, 1)
    z2   = ca1 @ W2                 16 matmuls
    ec5  = 0.69 + 0.3*sig(4*(ec5 + z2) - 1.2)
    ec3  = ec3*(P*ec5 - Q) + C      (input + noise-mask folded into P,Q,C)

Matmul operands are fp16 (fast weight load), accumulation fp32 in PSUM, state
fp32. drive (ca3 gaussian drive) and the P/Q/C affine coefficients are
precomputed on the host. The act head (512x2) is also applied on the host from
the ca1 history.
"""

import sys

sys.path.insert(0, "/opt/trn_rl_repo")

import numpy as np

import concourse.bass as bass
import concourse.mybir as mybir
import concourse.tile as tile
from concourse.bass_utils import run_bass_kernel_spmd

F32 = mybir.dt.float32
F16 = mybir.dt.float16
ALU = mybir.AluOpType
AFT = mybir.ActivationFunctionType

LINEARIZE = False
N_CORES = 8
BS = 128
BPC = BS // N_CORES  # 16 batch per core
NEC = 512
NBLK = NEC // 128  # 4 j-blocks
TS = 0.1
SIGMA = 5.0

_nc_cache = {}


def _legalize_waits(nc: bass.Bass) -> None:
    """Walrus allows only ONE sync wait per instruction (2 on EventSemaphore).
    Tile sometimes emits more (multi-queue DMA fanout, PSUM WAW + RAW joins).
    Spill excess waits onto same-engine InstNoOp instructions inserted right
    before the offender — program order on the engine enforces them."""
    import bass_rust as _br

    spill_id = [0]
    for f in nc.m.functions:
        for b in f.blocks:
            il = b.instructions
            changed = False
            newl = []
            for i in il:
                si = i.sync_info
                cap = 2 if isinstance(i, mybir.InstEventSemaphore) else 1
                if si is not None and len(si.on_wait) > cap:
                    waits = list(si.on_wait)
                    for w in waits[:-cap]:
                        nop = mybir.InstNoOp(
                            name=f"wait_spill_{spill_id[0]}", ins=[], outs=[]
                        )
                        spill_id[0] += 1
                        nop.engine = i.engine
                        nop.sync_info = _br.SyncInfo(on_wait=[w], on_update=[])
                        newl.append(nop)
                    i.sync_info = _br.SyncInfo(
                        on_wait=waits[-cap:], on_update=list(si.on_update)
                    )
                    changed = True
                newl.append(i)
            if changed:
                b.instructions = newl


def _build_nc(T: int) -> bass.Bass:
    nc = bass.Bass()

    w1 = nc.declare_dram_parameter("w1", [128, NBLK * NEC], F16, isOutput=False)
    w2 = nc.declare_dram_parameter("w2", [128, NBLK * NEC], F16, isOutput=False)
    d0 = nc.declare_dram_parameter("d0", [128, NBLK * T], F32, isOutput=False)
    d1 = nc.declare_dram_parameter("d1", [128, NBLK * T], F32, isOutput=False)
    ec3i = nc.declare_dram_parameter("ec3i", [128, 64], F32, isOutput=False)
    ec5i = nc.declare_dram_parameter("ec5i", [128, 64], F32, isOutput=False)
    pqc = nc.declare_dram_parameter("pqc", [T, 128, 192], F32, isOutput=False)

    ec3his = nc.declare_dram_parameter("ec3his", [T, 128, 64], F32, isOutput=True)
    ec5his = nc.declare_dram_parameter("ec5his", [T, 128, 64], F32, isOutput=True)
    ca1his = nc.declare_dram_parameter("ca1his", [T, 128, 64], F16, isOutput=True)
    ec3f = nc.declare_dram_parameter("ec3f", [128, 64], F32, isOutput=True)
    ec5f = nc.declare_dram_parameter("ec5f", [128, 64], F32, isOutput=True)

    with tile.TileContext(nc, linearize=LINEARIZE) as tc:
        with (
            tc.tile_pool(name="const", bufs=1) as cpool,
            tc.tile_pool(name="state", bufs=3) as spool,
            tc.tile_pool(name="work", bufs=2) as wpool,
            tc.tile_pool(name="io", bufs=4) as iopool,
            tc.tile_pool(name="ps", bufs=1, space="PSUM") as pspool,
        ):
            w1s = cpool.tile([128, NBLK * NEC], F16, tag="w1")
            nc.sync.dma_start(w1s[:], w1[:])
            w2s = cpool.tile([128, NBLK * NEC], F16, tag="w2")
            nc.sync.dma_start(w2s[:], w2[:])
            d0s = cpool.tile([128, NBLK * T], F32, tag="d0")
            nc.sync.dma_start(d0s[:], d0[:])
            d1s = cpool.tile([128, NBLK * T], F32, tag="d1")
            nc.sync.dma_start(d1s[:], d1[:])
            b12 = cpool.tile([128, 1], F32, tag="b12")
            nc.vector.memset(b12[:], -1.2)

            ec3i_t = spool.tile([128, 64], F32, tag="ec3raw")
            nc.sync.dma_start(ec3i_t[:], ec3i[:])
            ec5i_t = spool.tile([128, 64], F32, tag="ec5raw")
            nc.sync.dma_start(ec5i_t[:], ec5i[:])
            ec3h = spool.tile([128, 64], F16, tag="ec3h")
            nc.vector.tensor_copy(ec3h[:], ec3i_t[:])
            # Land the init states on DVE so every later consumer's dep is
            # same-engine (HW allows only ONE sync wait per instruction).
            ec3 = spool.tile([128, 64], F32, tag="ec3")
            nc.vector.tensor_copy(ec3[:], ec3i_t[:])
            ec5 = spool.tile([128, 64], F32, tag="ec5")
            nc.vector.tensor_copy(ec5[:], ec5i_t[:])

            # Absorb const-tile DMA waits (one warmup op per DMA, per engine).
            warm = cpool.tile([1, 2], F32, tag="warm")
            nc.vector.tensor_copy(warm[:, 0:1], d0s[0:1, 0:1])
            nc.vector.tensor_copy(warm[:, 1:2], d1s[0:1, 0:1])

            def view4(ap):  # [128, 4*512] psum -> [128, 4, 16] (m-chunk slices)
                return ap.rearrange("p (m f) -> p m f", m=4)[:, :, 0:16]

            # Dummy matmuls: absorb the w1/w2 DMA waits on the PE so the first
            # real matmul only waits on its rhs. Results land in unused psum
            # columns that a later start=True group re-zeroes.
            ps2w = pspool.tile([128, 4 * 512], F32, tag="ps2")
            nc.tensor.matmul(ps2w[0:1, 100:101], w1s[0:1, 0:1], w1s[0:1, 0:1],
                             start=True, stop=True, skip_group_check=True)
            nc.tensor.matmul(ps2w[0:1, 612:613], w2s[0:1, 0:1], w2s[0:1, 0:1],
                             start=True, stop=True, skip_group_check=True)

            for t in range(T):
                pqct = iopool.tile([128, 192], F32, tag="pqc")
                nc.sync.dma_start(pqct[:], pqc[t])
                pql = wpool.tile([128, 192], F32, tag="pql")
                nc.vector.tensor_copy(pql[:], pqct[:])

                ps1 = pspool.tile([128, 4 * 512], F32, tag="ps1")
                for m in range(4):
                    for k in range(4):
                        nc.tensor.matmul(
                            ps1[:, m * 512 : m * 512 + 16],
                            w1s[:, k * 512 + m * 128 : k * 512 + (m + 1) * 128],
                            ec3h[:, k * 16 : (k + 1) * 16],
                            start=(k == 0),
                            stop=(k == 3),
                        )
                sb1 = wpool.tile([128, 64], F32, tag="sb1")
                nc.vector.tensor_copy(
                    sb1[:].rearrange("p (m f) -> p m f", m=4), view4(ps1[:])
                )
                sig = wpool.tile([128, 64], F32, tag="sig")
                nc.scalar.activation(sig[:], sb1[:], AFT.Sigmoid)
                u = wpool.tile([128, 64], F32, tag="u")
                for jb in range(4):
                    col = jb * T + t
                    nc.vector.tensor_scalar(
                        u[:, jb * 16 : (jb + 1) * 16],
                        sig[:, jb * 16 : (jb + 1) * 16],
                        d0s[:, col : col + 1],
                        d1s[:, col : col + 1],
                        ALU.mult,
                        ALU.add,
                    )
                ca1h = wpool.tile([128, 64], F16, tag="ca1h")
                nc.vector.tensor_scalar(
                    ca1h[:], u[:], 0.0, 1.0, ALU.max, ALU.min
                )
                stc = iopool.tile([128, 64], F16, tag="stc")
                nc.vector.tensor_copy(stc[:], ca1h[:])
                nc.sync.dma_start(ca1his[t], stc[:])

                ps2 = pspool.tile([128, 4 * 512], F32, tag="ps2")
                for m in range(4):
                    for k in range(4):
                        nc.tensor.matmul(
                            ps2[:, m * 512 : m * 512 + 16],
                            w2s[:, k * 512 + m * 128 : k * 512 + (m + 1) * 128],
                            ca1h[:, k * 16 : (k + 1) * 16],
                            start=(k == 0),
                            stop=(k == 3),
                        )
                e = wpool.tile([128, 64], F32, tag="e")
                nc.vector.tensor_tensor(
                    e[:].rearrange("p (m f) -> p m f", m=4), view4(ps2[:]),
                    ec5[:].rearrange("p (m f) -> p m f", m=4), ALU.add,
                )
                s5 = wpool.tile([128, 64], F32, tag="s5")
                nc.scalar.activation(s5[:], e[:], AFT.Sigmoid, bias=b12[:], scale=4.0)
                ec5n = spool.tile([128, 64], F32, tag="ec5")
                nc.vector.tensor_scalar(
                    ec5n[:], s5[:], 0.3, 0.69, ALU.mult, ALU.add
                )
                st5 = iopool.tile([128, 64], F32, tag="st5")
                nc.vector.tensor_copy(st5[:], ec5n[:])
                nc.sync.dma_start(ec5his[t], st5[:])

                t1 = wpool.tile([128, 64], F32, tag="t1")
                nc.vector.tensor_tensor(t1[:], ec5n[:], pql[:, 0:64], ALU.mult)
                t2 = wpool.tile([128, 64], F32, tag="t2")
                nc.vector.tensor_tensor(t2[:], t1[:], pql[:, 64:128], ALU.subtract)
                t3 = wpool.tile([128, 64], F32, tag="t3")
                nc.vector.tensor_tensor(t3[:], ec3[:], t2[:], ALU.mult)
                ec3n = spool.tile([128, 64], F32, tag="ec3")
                nc.vector.tensor_tensor(ec3n[:], t3[:], pql[:, 128:192], ALU.add)
                st3 = iopool.tile([128, 64], F32, tag="st3")
                nc.vector.tensor_copy(st3[:], ec3n[:])
                nc.sync.dma_start(ec3his[t], st3[:])
                ec3hn = spool.tile([128, 64], F16, tag="ec3h")
                nc.vector.tensor_copy(ec3hn[:], ec3n[:])

                ec3, ec5, ec3h = ec3n, ec5n, ec3hn

            nc.sync.dma_start(ec3f[:], ec3[:])
            nc.sync.dma_start(ec5f[:], ec5[:])

    _legalize_waits(nc)
    return nc


def _run_pjrt_timed(nc, in_maps, n_cores, reps=3):
    """Replicates bass2jax.run_bass_via_pjrt's multi-core path, but keeps the
    jitted executable + device-resident inputs so the NEFF can be re-executed
    and wall-clock timed (the axon NTFF profile hook is unavailable here)."""
    import time as _time

    import jax
    from jax.experimental.shard_map import shard_map
    from jax.sharding import Mesh, NamedSharding, PartitionSpec

    from concourse import bass2jax

    bass2jax.install_neuronx_cc_hook()
    assert nc.dbg_addr is None
    partition_name = nc.partition_id_tensor.name if nc.partition_id_tensor else None

    in_names, out_names, out_avals, zero_outs = [], [], [], []
    for alloc in nc.m.functions[0].allocations:
        if not isinstance(alloc, mybir.MemoryLocationSet):
            continue
        name = alloc.memorylocations[0].name
        if alloc.kind == "ExternalInput":
            if name != partition_name:
                in_names.append(name)
        elif alloc.kind == "ExternalOutput":
            out_names.append(name)
            shape = tuple(alloc.tensor_shape)
            dtype = mybir.dt.np(alloc.dtype)
            out_avals.append(jax.core.ShapedArray(shape, dtype))
            zero_outs.append(np.zeros(shape, dtype))
    n_params = len(in_names)
    all_names = in_names + out_names
    if partition_name is not None:
        all_names = all_names + [partition_name]

    def _body(*args):
        operands = list(args)
        if partition_name is not None:
            operands.append(bass2jax.partition_id_tensor())
        outs = bass2jax._bass_exec_p.bind(
            *operands,
            out_avals=tuple(out_avals),
            in_names=tuple(all_names),
            out_names=tuple(out_names),
            lowering_input_output_aliases=(),
            sim_require_finite=True,
            sim_require_nnan=True,
            nc=nc,
        )
        return tuple(outs)

    devices = jax.devices()[:n_cores]
    mesh = Mesh(np.asarray(devices), ("core",))
    nshard = NamedSharding(mesh, PartitionSpec("core"))
    in_specs = (PartitionSpec("core"),) * (n_params + len(out_names))
    out_specs = (PartitionSpec("core"),) * len(out_names)
    fn = jax.jit(
        shard_map(_body, mesh=mesh, in_specs=in_specs, out_specs=out_specs,
                  check_rep=False),
        keep_unused=True,
    )
    concat_in = [
        np.concatenate([np.asarray(in_maps[c][nm]) for c in range(n_cores)], axis=0)
        for nm in in_names
    ]
    dev_in = [jax.device_put(a, nshard) for a in concat_in]
    dev_zero = [
        jax.device_put(np.zeros((n_cores * z.shape[0], *z.shape[1:]), z.dtype), nshard)
        for z in zero_outs
    ]
    outs = fn(*dev_in, *dev_zero)
    jax.block_until_ready(outs)
    times = []
    for _ in range(reps):
        t0 = _time.perf_counter()
        o = fn(*dev_in, *dev_zero)
        jax.block_until_ready(o)
        times.append(_time.perf_counter() - t0)
    results = [
        {
            nm: np.asarray(outs[i]).reshape(n_cores, *out_avals[i].shape)[c]
            for i, nm in enumerate(out_names)
        }
        for c in range(n_cores)
    ]
    return results, (min(times) if times else None)


def _to_blocked(x_t):
    """(N, B) with N=512 -> [128, NBLK*B] blocked tile layout."""
    n, b = x_t.shape
    return np.ascontiguousarray(
        x_t.reshape(NBLK, 128, b).transpose(1, 0, 2).reshape(128, NBLK * b)
    )


def _from_blocked_his(h):
    """(T, 128, 64) device layout -> (BPC, T, 512)."""
    T = h.shape[0]
    return np.ascontiguousarray(
        h.reshape(T, 128, NBLK, BPC).transpose(3, 0, 2, 1).reshape(BPC, T, NEC)
    )


def _from_blocked_state(s):
    """(128, 64) -> (BPC, 512)."""
    return np.ascontiguousarray(
        s.reshape(128, NBLK, BPC).transpose(2, 1, 0).reshape(BPC, NEC)
    )


def kernel(ec3input, ec3_last, ec5_last, ca1_last, ca1bias, wca3ca1,
           wec3ca1, wca1ec5, wca1act, actbias, noise_mask, ca3order,
           timing_reps=0):
    ec3input = np.asarray(ec3input, dtype=np.float32)
    ec3_last = np.asarray(ec3_last, dtype=np.float32)
    ec5_last = np.asarray(ec5_last, dtype=np.float32)
    ca1bias = np.asarray(ca1bias, dtype=np.float32)
    wca3ca1 = np.asarray(wca3ca1, dtype=np.float32)
    wec3ca1 = np.asarray(wec3ca1, dtype=np.float32)
    wca1ec5 = np.asarray(wca1ec5, dtype=np.float32)
    wca1act = np.asarray(wca1act, dtype=np.float32)
    actbias = np.asarray(actbias, dtype=np.float32)
    mask = np.asarray(noise_mask)
    order = np.asarray(ca3order)

    bs, T, _ = ec3input.shape
    assert bs == BS

    # --- host precompute: ca3 drive per step ---------------------------------
    ca3num = wca3ca1.shape[0]
    centers = np.linspace(-0.1 * T, 1.1 * T, ca3num, dtype=np.float32)[order]
    xs_t = np.arange(T, dtype=np.float32)
    ca3 = np.exp(-((centers[None, :] - xs_t[:, None]) ** 2) / (SIGMA**2) / 2.0)
    drive = (ca3.astype(np.float32) @ wca3ca1).astype(np.float32)  # (T, 512)

    d0 = _to_blocked(drive.T)  # [128, 4T] : d0[p, jb*T + t]
    d1 = _to_blocked((drive - ca1bias[None, :]).T)

    w1 = _to_blocked(wec3ca1).astype(np.float16)  # [128, 4*512]
    w2 = _to_blocked(wca1ec5).astype(np.float16)

    # --- host precompute: P/Q/C mask+input affine coefficients ----------------
    # ec3_new = ec3*(P*ec5 - Q) + C;  P = 1-0.5m, Q = P*xs, C = Q_ + 0.3m
    # with xs = 0.6 * x_in
    key = (T,)
    if key not in _nc_cache:
        _nc_cache[key] = _build_nc(T)
    nc = _nc_cache[key]

    in_maps = []
    for c in range(N_CORES):
        sl = slice(c * BPC, (c + 1) * BPC)
        x = ec3input[sl]  # (16, T, 512)
        m = mask[sl].astype(np.float32)
        xs = 0.6 * x
        P = 1.0 - 0.5 * m
        Q = P * xs
        C = Q + 0.3 * m

        def to_dev(a):  # (16, T, 512) -> (T, 128, 64)
            return np.ascontiguousarray(
                a.transpose(1, 2, 0)
                .reshape(T, NBLK, 128, BPC)
                .transpose(0, 2, 1, 3)
                .reshape(T, 128, NBLK * BPC)
            )

        pqc = np.concatenate(
            [to_dev(P), to_dev(Q), to_dev(C)], axis=2
        )  # (T, 128, 192)
        in_maps.append(
            {
                "w1": w1,
                "w2": w2,
                "d0": d0,
                "d1": d1,
                "ec3i": _to_blocked(ec3_last[sl].T),
                "ec5i": _to_blocked(ec5_last[sl].T),
                "pqc": np.ascontiguousarray(pqc),
            }
        )

    if timing_reps:
        results, t_min = _run_pjrt_timed(nc, in_maps, N_CORES, reps=timing_reps)
    else:
        results = run_bass_kernel_spmd(nc, in_maps, list(range(N_CORES))).results
        t_min = None

    # --- gather + untranspose -------------------------------------------------
    actlist = np.empty((BS, T, wca1act.shape[1]), np.float32)
    ec3his = np.empty((BS, T, NEC), np.float32)
    ec5his = np.empty((BS, T, NEC), np.float32)
    ca1his = np.empty((BS, T, NEC), np.float32)
    ec3fin = np.empty((BS, NEC), np.float32)
    ec5fin = np.empty((BS, NEC), np.float32)
    for c in range(N_CORES):
        sl = slice(c * BPC, (c + 1) * BPC)
        r = results[c]
        ec3his[sl] = _from_blocked_his(r["ec3his"])
        ec5his[sl] = _from_blocked_his(r["ec5his"])
        ca1his[sl] = _from_blocked_his(r["ca1his"].astype(np.float32))
        ec3fin[sl] = _from_blocked_state(r["ec3f"])
        ec5fin[sl] = _from_blocked_state(r["ec5f"])

    # act head on host from ca1 history
    np.matmul(ca1his, wca1act, out=actlist)
    actlist += actbias[None, None, :]
    ca1fin = ca1his[:, -1, :].copy()

    kernel.last_time_s = t_min
    return (actlist, ec3his, ec5his, ca1his, ec3fin, ec5fin, ca1fin)
